# revision 1
# baseline (speedup 1.0000x reference)
"""Trainium2 Bass kernel for nn_Attention2D (sparse_attention).

Self-contained: takes FULL unsharded inputs, shards data-parallel over the
leading (n_rays) axis across 8 NeuronCores, runs a fused Bass/Tile kernel per
core, gathers the full output.

Math (validated against the jax reference to ~2.6e-6 abs):
  s cancels in kh - qh, so with host-precomputed
    A_k = Wk.T@attn_w1, A_q = Wq.T@attn_w1, P_a = pos_w2@attn_w1,
    c_z = pos_b2@attn_w1 + attn_b1
  the attn-MLP hidden is h1 = relu(k@A_k - q@A_q + hpos@P_a + c_z) with
  hpos = relu(pos@pos_w1 + pos_b1).  The mask is carried through the pipeline
  as an extra matmul row (relu(m)=m for m in {0,1}); masked tokens get their
  h1 clipped to 0 via a +50*(m-1) rank-1 term, and the logits get a +50*m
  rank-1 shift so that after exp(logit-50) masked entries are exp(-50)≈2e-22
  (nonzero => all-masked rays reproduce the reference's uniform softmax).
  u = vh + p (its bias s+pos_b2 is folded into the output bias since softmax
  weights sum to 1), x = (sum_v u*e)/(sum_v e), out = x@out_w + out_b'.

Layout: all on-chip activations are feature-major [channel, token]; the host
pre-transposes k/q/pos/mask into per-core contiguous arrays so every DMA is
dense, and un-transposes the [channel-major] output at the end.
"""

import numpy as np
import ml_dtypes

BF16 = ml_dtypes.bfloat16
DIM, HID, B, N, V = 64, 8, 1024, 64, 8
NCORES = 8
B_C = B // NCORES          # 128 b-rows per core
R_C = B_C * N              # 8192 rays per core
T_C = R_C * V              # 65536 view-tokens per core
TILE_T = 1024              # tokens per tile
HT = TILE_T // 2           # 512  (half-tile tokens; L1 free size)
QT = TILE_T // 4           # 256  (quarter-tile tokens; L2 free size)
RH = TILE_T // 16          # 64   (rays per half-tile)
NT_FULL = T_C // TILE_T    # 64 tiles per core
GRP = 16                   # tiles per pm/out DMA group
M_SHIFT = 50.0             # logit shift for masked-softmax trick
CLIP = 50.0                # relu clipping magnitude for masked tokens

# consts tensor column layout
C_WVT, C_AK, C_AQN, C_WP5, C_WHQ, C_PW2, C_W3, C_OW = 0, 64, 96, 128, 160, 192, 256, 320
C_BHP, C_BH1, C_BOUT, C_BEXP = 384, 385, 386, 387
CW = 388

_PROG_CACHE: dict = {}


# ----------------------------------------------------------------------------
# host-side preparation
# ----------------------------------------------------------------------------

def _f32(x):
    return np.ascontiguousarray(np.asarray(x), dtype=np.float32)


def make_consts(inputs) -> np.ndarray:
    """Build the [128, CW] constants array (shared by all cores)."""
    eid = int(np.asarray(inputs["embed_id1"]))
    Wq = _f32(inputs["q_tbl"])[eid].reshape(DIM, DIM)
    Wk = _f32(inputs["k_tbl"])[eid].reshape(DIM, DIM)
    Wv = _f32(inputs["v_tbl"])[eid].reshape(DIM, DIM)
    pos_w1, pos_b1 = _f32(inputs["pos_w1"]), _f32(inputs["pos_b1"])
    pos_w2, pos_b2 = _f32(inputs["pos_w2"]), _f32(inputs["pos_b2"])
    attn_w1, attn_b1 = _f32(inputs["attn_w1"]), _f32(inputs["attn_b1"])
    attn_w2, attn_b2 = _f32(inputs["attn_w2"]), _f32(inputs["attn_b2"])
    out_w, out_b = _f32(inputs["out_w"]), _f32(inputs["out_b"])
    str_w, str_b = _f32(inputs["str_w"]), _f32(inputs["str_b"])
    strength = _f32(inputs["strength"])

    s = strength @ str_w + str_b                  # [64]
    A_k = Wk.T @ attn_w1                          # [64, 8]
    A_q = Wq.T @ attn_w1                          # [64, 8]
    P_a = pos_w2 @ attn_w1                        # [8, 8]
    c_z = pos_b2 @ attn_w1 + attn_b1              # [8]
    sb2 = s + pos_b2                              # [64]
    out_b_p = sb2 @ out_w + out_b                 # [64]

    C = np.zeros((128, CW), np.float32)
    # Wv.T stacked for both halves (lhsT of u matmul: [in-chan, out-chan])
    C[0:64, C_WVT:C_WVT + 64] = Wv.T
    C[64:128, C_WVT:C_WVT + 64] = Wv.T
    # combined K=128 lhsT for the merged kA-qA matmul:
    # C_AK block pairs with kqA tiles (k-half-A rows 0-63, q-bcast rows 64-127)
    # C_AQN block pairs with kqB tiles (q-bcast rows 0-63, k-half-B rows 64-127)
    C[0:64, C_AK:C_AK + 8] = A_k
    C[64:128, C_AK:C_AK + 8] = -A_q
    C[0:64, C_AQN:C_AQN + 8] = -A_q
    C[64:128, C_AQN:C_AQN + 8] = A_k
    for qq in range(4):
        r = 32 * qq
        # pos-MLP stage1 lhsT [5, 32]: rows 0-3 pos_w1 -> cols 0:8 ; mask row
        # 4 -> col 8 (carries mask into hpos row 8)
        C[r:r + 4, C_WP5:C_WP5 + 8] = pos_w1
        C[r + 4, C_WP5 + 8] = 1.0
        # z_pre stage lhsT [9, 32]: rows 0-7 = P_a -> cols 0:8, mask row 8 ->
        # +CLIP on all 9 outputs
        C[r:r + 8, C_WHQ:C_WHQ + 8] = P_a
        C[r + 8, C_WHQ:C_WHQ + 10] = CLIP
        # pos_w2 [8, 64] for u accumulation
        C[r:r + 8, C_PW2:C_PW2 + 64] = pos_w2
        # W3 [10, 64]: attn_w2 rows + bias row + exact +50 shift row
        C[r:r + 8, C_W3:C_W3 + 64] = attn_w2
        C[r + 8, C_W3:C_W3 + 64] = attn_b2
        C[r + 9, C_W3:C_W3 + 64] = M_SHIFT
        # biases (per-partition vectors)
        C[r:r + 8, C_BHP] = pos_b1
        C[r + 8, C_BHP] = 0.0
        C[r:r + 8, C_BH1] = c_z - CLIP
        C[r + 8, C_BH1] = 1.0 - CLIP
        C[r + 9, C_BH1] = 1.0 - CLIP
    # out_w stacked; out bias per channel stacked
    for h in range(2):
        C[64 * h:64 * h + 64, C_OW:C_OW + 64] = out_w
        C[64 * h:64 * h + 64, C_BOUT] = out_b_p
    C[:, C_BEXP] = -M_SHIFT
    return C


def prep_core(q, k, pos, mask_f, core, nt=NT_FULL):
    """Per-core transposed contiguous arrays. q/k/pos/mask_f are full arrays."""
    ntok = nt * TILE_T
    nray = ntok // V
    b0 = core * B_C
    kc = _f32(k[b0:b0 + B_C]).reshape(T_C, DIM)[:ntok]
    qc = _f32(q[b0:b0 + B_C]).reshape(R_C, DIM)[:nray]
    pc = _f32(pos[b0:b0 + B_C]).reshape(T_C, 4)[:ntok]
    mc = mask_f[b0:b0 + B_C].reshape(T_C)[:ntok]

    # k channel-major halves + per-view-replicated q, combined per half so the
    # merged K=128 kA-qA matmul can stream one tile:
    #   kqA rows 0-63 = k-half-A channels, rows 64-127 = q-half-A repeated x8
    #   kqB rows 0-63 = q-half-B repeated x8, rows 64-127 = k-half-B channels
    kT = kc.reshape(nt, 2, HT, DIM).transpose(1, 3, 0, 2).reshape(128, nt * HT)
    qT = qc.reshape(nt, 2, RH, DIM).transpose(1, 3, 0, 2).reshape(128, nt * RH)
    qrep = np.repeat(qT, V, axis=1)              # [128, nt*HT]
    kqA = np.ascontiguousarray(
        np.concatenate([kT[0:64], qrep[0:64]], axis=0).astype(BF16))
    kqB = np.ascontiguousarray(
        np.concatenate([qrep[64:128], kT[64:128]], axis=0).astype(BF16))
    # posm [20, nt*QT]: row qq*5+e (e<4: pos feat, e=4: mask), col t*QT+j
    pm4 = pc.reshape(nt, 4, QT, 4).transpose(1, 3, 0, 2)        # [4(qq),4(e),nt,QT]
    m4 = mc.reshape(nt, 4, QT).transpose(1, 0, 2)               # [4(qq),nt,QT]
    posm = np.ascontiguousarray(
        np.concatenate([pm4, m4[:, None]], axis=1).reshape(20, nt * QT)
        .astype(BF16))
    return {"kqA": kqA, "kqB": kqB, "posm": posm}


def unprep_out(outT, nt=NT_FULL):
    """outT [128, nt*RH] channel-major -> [nt*2*RH, 64] token-major."""
    v = outT.reshape(2, 64, nt, RH).transpose(2, 0, 3, 1)
    return np.ascontiguousarray(v.reshape(nt * 2 * RH, DIM))


# ----------------------------------------------------------------------------
# device program
# ----------------------------------------------------------------------------

def build_program(nt=NT_FULL, nrep=1, skip=""):
    """Build + compile the per-core Bass program (cached)."""
    if (nt, nrep, skip) in _PROG_CACHE:
        return _PROG_CACHE[(nt, nrep, skip)]

    import concourse.bacc as bacc
    import concourse.tile as tile
    import concourse.mybir as mybir

    f32 = mybir.dt.float32
    bf16 = mybir.dt.bfloat16
    nc = bacc.Bacc("TRN2", target_bir_lowering=False, debug=False,
                   enable_asserts=False, num_devices=NCORES)
    kqA_d = nc.dram_tensor("kqA", [128, nt * HT], bf16, kind="ExternalInput").ap()
    kqB_d = nc.dram_tensor("kqB", [128, nt * HT], bf16, kind="ExternalInput").ap()
    posm_d = nc.dram_tensor("posm", [20, nt * QT], bf16, kind="ExternalInput").ap()
    cons_d = nc.dram_tensor("consts", [128, CW], bf16, kind="ExternalInput").ap()
    bias_d = nc.dram_tensor("biasc", [128, 4], f32, kind="ExternalInput").ap()
    outT_d = nc.dram_tensor("outT", [128, nt * RH], f32, kind="ExternalOutput").ap()

    with tile.TileContext(nc) as tc:
        _emit(tc, nc, mybir, kqA_d, kqB_d, posm_d, cons_d, bias_d, outT_d, nt, nrep, skip)
    nc.compile()
    _PROG_CACHE[(nt, nrep, skip)] = nc
    return nc


def _emit(tc, nc, mybir, kqA_d, kqB_d, posm_d, cons_d, bias_d, outT_d, nt, nrep=1, skip_str=""):
    from contextlib import ExitStack
    skip = set(skip_str.split(","))

    f32 = mybir.dt.float32
    Relu = mybir.ActivationFunctionType.Relu
    Exp = mybir.ActivationFunctionType.Exp
    Ident = mybir.ActivationFunctionType.Identity
    mult = mybir.AluOpType.mult
    AX = mybir.AxisListType.X
    grp = min(GRP, nt)
    bf16 = mybir.dt.bfloat16
    r32 = lambda ap: ap


    with ExitStack() as ctx:
        ep = ctx.enter_context
        cpool = ep(tc.tile_pool(name="consts", bufs=1))
        kpool = ep(tc.tile_pool(name="kt", bufs=2))
        pmpool = ep(tc.tile_pool(name="pm", bufs=2))
        qpool = ep(tc.tile_pool(name="qt", bufs=2))
        hpool = ep(tc.tile_pool(name="hid", bufs=3))
        epool = ep(tc.tile_pool(name="east", bufs=2))
        tpool = ep(tc.tile_pool(name="tprod", bufs=2))
        spool = ep(tc.tile_pool(name="small", bufs=4))
        opool = ep(tc.tile_pool(name="ob", bufs=2))
        pp_h = ep(tc.tile_pool(name="ps_h", bufs=1, space="PSUM"))
        pp_z = ep(tc.tile_pool(name="ps_z", bufs=1 if "bufs2" not in skip else 2,
                               space="PSUM"))
        pp_u = ep(tc.tile_pool(name="ps_u", bufs=1, space="PSUM"))
        pp_l = ep(tc.tile_pool(name="ps_l", bufs=2, space="PSUM"))

        cons = cpool.tile([128, CW], bf16, tag="consts")
        nc.sync.dma_start(cons[:], cons_d[:, :])
        biasc = cpool.tile([128, 4], f32, tag="biasc")
        nc.sync.dma_start(biasc[:], bias_d[:, :])
        b_hp = biasc[:, 0:1]
        b_h1 = biasc[:, 1:2]
        b_out = biasc[:, 2:3]
        b_exp = biasc[:, 3:4]

        for rep in range(nrep):
         for g in range((nt + grp - 1) // grp):
            gt = min(grp, nt - g * grp)
            pm = pmpool.tile([128, grp * QT], bf16, tag="pm")
            for qq in range(4):
                nc.sync.dma_start(
                    pm[32 * qq:32 * qq + 5, 0:gt * QT],
                    posm_d[5 * qq:5 * qq + 5, g * grp * QT:g * grp * QT + gt * QT])
            ob = opool.tile([128, grp * RH], f32, tag="ob")

            for ti in range(gt):
                t = g * grp + ti
                if ti % 8 == 0:
                    kqa_b = kpool.tile([128, 8 * HT], bf16, tag="kqa")
                    kqb_b = kpool.tile([128, 8 * HT], bf16, tag="kqb")
                    nb = min(8, gt - ti)
                    nc.sync.dma_start(kqa_b[:, 0:nb * HT],
                                      kqA_d[:, t * HT:t * HT + nb * HT])
                    nc.sync.dma_start(kqb_b[:, 0:nb * HT],
                                      kqB_d[:, t * HT:t * HT + nb * HT])
                off = (ti % 8) * HT
                kqa = kqa_b[:, off:off + HT]
                kqb = kqb_b[:, off:off + HT]

                pmt = pm[:, ti * QT:(ti + 1) * QT]

                # ---- pos-MLP stage 1 (+ mask carried into hpos row 8) ----
                hpos_ps = pp_h.tile([128, QT], f32, tag="hps")
                for qq in range(4):
                    r = 32 * qq
                    nc.tensor.matmul(
                        hpos_ps[r:r + 32, :], r32(cons[r:r + 5, C_WP5:C_WP5 + 32]),
                        r32(pmt[r:r + 5, :]), start=True, stop=True,
                        tile_position=(r, r), skip_group_check=True)
                hpos = hpool.tile([128, QT], bf16, tag="hpos")
                nc.scalar.activation(hpos[:], hpos_ps[:], Relu, bias=b_hp)

                # ---- z_pre accumulation: (kA - qA) via one K=128 matmul ----
                z_ps = pp_z.tile([128, QT], f32, tag="zps")
                for qq in range(4):
                    r, h, f = 32 * qq, qq // 2, qq % 2
                    kq = kqa if h == 0 else kqb
                    cblk = C_AK if h == 0 else C_AQN
                    nc.tensor.matmul(
                        z_ps[r:r + 32, :], r32(cons[:, cblk:cblk + 32]),
                        r32(kq[:, f * QT:(f + 1) * QT]),
                        start=True, stop=False, tile_position=(0, r),
                        skip_group_check=True)
                for qq in range(4):
                    r = 32 * qq
                    nc.tensor.matmul(
                        z_ps[r:r + 32, :], r32(cons[r:r + 9, C_WHQ:C_WHQ + 32]),
                        r32(hpos[r:r + 9, :]), start=False, stop=True,
                        tile_position=(r, r), skip_group_check=True)
                h1 = hpool.tile([128, QT], bf16, tag="h1")
                nc.scalar.activation(h1[:], z_ps[:], Relu, bias=b_h1)

                # ---- logits = h1m @ [attn_w2; attn_b2; 50] ----
                # two PSUM banks (f=0 at cols 0:256, f=1 at cols 512:768) so
                # concurrent row-group matmuls never drain into the same
                # (partition-range, bank) pair -- that combination hangs HW.
                lg_ps = pp_l.tile([128, 2 * HT], f32, tag="lps")
                for qq in range(4):
                    r, h, f = 32 * qq, qq // 2, qq % 2
                    nc.tensor.matmul(
                        lg_ps[64 * h:64 * h + 64, f * HT:f * HT + QT],
                        r32(cons[r:r + 10, C_W3:C_W3 + 64]), r32(h1[r:r + 10, :]),
                        start=True, stop=True, tile_position=(r, 64 * h),
                        skip_group_check=True)

                # ---- u = k@Wv.T + hpos@pos_w2 ----
                u_ps = pp_u.tile([128, 2 * HT], f32, tag="ups")
                for h in range(2):
                    kq = kqa if h == 0 else kqb
                    for f in range(2):
                        nc.tensor.matmul(
                            u_ps[64 * h:64 * h + 64, f * HT:f * HT + QT],
                            r32(cons[64 * h:64 * h + 64, C_WVT:C_WVT + 64]),
                            r32(kq[64 * h:64 * h + 64, f * QT:(f + 1) * QT]),
                            start=True, stop=False,
                            tile_position=(64 * h, 64 * h),
                            skip_group_check=True)
                for qq in range(4):
                    r, h, f = 32 * qq, qq // 2, qq % 2
                    nc.tensor.matmul(
                        u_ps[64 * h:64 * h + 64, f * HT:f * HT + QT],
                        r32(cons[r:r + 8, C_PW2:C_PW2 + 64]), r32(hpos[r:r + 8, :]),
                        start=False, stop=True, tile_position=(r, 64 * h),
                        skip_group_check=True)

                # ---- softmax over views (groups of 8 along free axis) ----
                east = epool.tile([128, HT], f32, tag="east")
                lg_v = lg_ps[:].rearrange("p (b k) -> p b k", b=2)[:, :, 0:QT]
                if "exp" not in skip:
                    nc.scalar.activation(
                        east[:].rearrange("p (b k) -> p b k", b=2), lg_v, Exp,
                        bias=b_exp)
                if "dve" not in skip:
                    gsum = spool.tile([128, RH], f32, tag="gsum")
                    tp = tpool.tile([128, HT], f32, tag="tp")
                    xr = spool.tile([128, RH], f32, tag="xr")
                    rg = spool.tile([128, RH], f32, tag="rg")
                    xx = spool.tile([128, RH], bf16, tag="xx")
                    nc.vector.reduce_sum(
                        gsum[:], east[:].rearrange("p (r v) -> p r v", v=V), axis=AX)
                    u_v = u_ps[:].rearrange("p (b k) -> p b k", b=2)[:, :, 0:QT]
                    if "tmul" not in skip:
                        nc.vector.tensor_tensor(
                            tp[:].rearrange("p (b k) -> p b k", b=2), u_v,
                            east[:].rearrange("p (b k) -> p b k", b=2), mult)
                    nc.vector.reduce_sum(
                        xr[:], tp[:].rearrange("p (r v) -> p r v", v=V), axis=AX)
                    if "recip" not in skip:
                        nc.vector.reciprocal_approx_fast(rg[:], gsum[:])
                    nc.vector.tensor_tensor(xx[:], xr[:], rg[:], mult)

                # ---- out = x @ out_w + out_b' (channel-major) ----
                if "dve" in skip:
                    xx = spool.tile([128, RH], bf16, tag="xx")
                    src_e = east[:, 0:RH] if "exp" not in skip else h1[:, 0:RH]
                    nc.vector.tensor_copy(xx[:], src_e)
                # out-MM uses lg_ps's unused bank-0 columns: its writers are
                # same-position as the z3 matmuls there (serial, hazard-free),
                # and the freed banks double-buffer lg_ps.
                o_ps = lg_ps[:, QT:QT + RH]
                for h in range(2):
                    nc.tensor.matmul(
                        o_ps[64 * h:64 * h + 64, :],
                        cons[64 * h:64 * h + 64, C_OW:C_OW + 64],
                        xx[64 * h:64 * h + 64, :], start=True, stop=True,
                        tile_position=(64 * h, 64 * h), skip_group_check=True)
                nc.scalar.activation(ob[:, ti * RH:(ti + 1) * RH], o_ps[:],
                                     Ident, bias=b_out)

            nc.sync.dma_start(
                outT_d[:, g * grp * RH:g * grp * RH + gt * RH],
                ob[:, 0:gt * RH])


# ----------------------------------------------------------------------------
# entry point
# ----------------------------------------------------------------------------

def kernel(q, k, pos, strength, q_tbl, k_tbl, v_tbl,
           pos_w1, pos_b1, pos_w2, pos_b2,
           attn_w1, attn_b1, attn_w2, attn_b2,
           out_w, out_b, str_w, str_b, mask, embed_id1) -> np.ndarray:
    from concourse.bass_utils import run_bass_kernel_spmd

    inputs = dict(q=q, k=k, pos=pos, strength=strength, q_tbl=q_tbl,
                  k_tbl=k_tbl, v_tbl=v_tbl, pos_w1=pos_w1, pos_b1=pos_b1,
                  pos_w2=pos_w2, pos_b2=pos_b2, attn_w1=attn_w1,
                  attn_b1=attn_b1, attn_w2=attn_w2, attn_b2=attn_b2,
                  out_w=out_w, out_b=out_b, str_w=str_w, str_b=str_b,
                  mask=mask, embed_id1=embed_id1)
    nc = build_program(NT_FULL)
    consts_f = make_consts(inputs)
    consts = consts_f.astype(BF16)
    biasc = np.ascontiguousarray(
        consts_f[:, [C_BHP, C_BH1, C_BOUT, C_BEXP]], dtype=np.float32)
    mask_f = np.asarray(mask).astype(np.float32)
    in_maps = []
    for c in range(NCORES):
        m = prep_core(inputs["q"], inputs["k"], inputs["pos"], mask_f, c)
        m["consts"] = consts
        m["biasc"] = biasc
        in_maps.append(m)
    res = run_bass_kernel_spmd(nc, in_maps, core_ids=list(range(NCORES)))
    out = np.empty((B * N, DIM), np.float32)
    for c in range(NCORES):
        out[c * R_C:(c + 1) * R_C] = unprep_out(res.results[c]["outT"])
    return out.reshape(B, N, DIM)

